# revision 2
# baseline (speedup 1.0000x reference)
"""Trainium2 Bass kernel for the detection loss balancer (nn_Balancer).

Computes: sum(loss * min(12*cnt+1, 13)) / (B*H*W)
where cnt[r,c] counts boxes covering pixel (r,c); min(12cnt+1,13) is the
fg/bg weight (13 inside the union mask, 1 outside).

Strategy (data-parallel over batch, 1 image per NeuronCore):
  - Rectangle rasterization via TensorEngine: cnt is separable, so a K=128
    matmul of signed row/col compare banks accumulates 12*cnt onto a K=1
    ones-bias fill (PSUM = 12*cnt + 1), one PSUM tile per 120-row tile.
  - The weighted reduction out = (cnt' min 13) * loss with per-partition
    accum runs as scalar_tensor_tensor pieces split across DVE (reads PSUM
    directly) and GPSIMD/Pool (SBUF-only: the ACT engine pre-copies cnt'
    tiles to SBUF off the critical path). Piece sizes/engines are tuned
    against TimelineSim so the last loss chunk's data is consumed by both
    engines in parallel the moment its DMA sem fires.
  - DMA structure: the HWDGE issue pipe carries ONLY the loss chunks (the
    bounds load and the acc store ride SWDGE prep+trigger on Pool, skipping
    the 625ns HWDGE slot + 650ns DGE latency). The bounds gather's
    descriptors are prepped from t=0 and triggered ~1.4us in, landing the
    bounds transfer in front of the serialized loss-transfer stream.
  - Host sums the 8 cores' acc columns and divides by B*H*W.
"""

import numpy as np

try:
    import concourse.bass as bass
except ImportError:  # pragma: no cover - fallback for bare containers
    import sys
    for p in ("/opt/trn_rl_repo", "/root/.axon_site/_ro/trn_rl_repo"):
        if p not in sys.path:
            sys.path.insert(0, p)
    import concourse.bass as bass  # noqa: F401

import concourse.bacc as bacc
import concourse.mybir as mybir
from concourse.tile import TileContext
from concourse.bass_utils import run_bass_kernel_spmd

B, H, W = 8, 480, 640
DOWNSAMPLE = 4
FG_WEIGHT = 13.0
BG_WEIGHT = 1.0
N_CORES = 8

RP = 120                      # rows per tile; 480 = 4*120
N_RT = H // RP                # 4 row tiles

# --- schedule knobs (tuned against TimelineSim) ---
# Loss DMA chunks (row_tile, col0, col1) on the SP HWDGE queue, in order.
LOSS_CHUNKS = [(0, 0, 640), (1, 0, 640), (2, 0, 640), (3, 0, 640)]
# SP nops before the first loss issue so the bounds gather-trigger wins the
# DMA_ENGINES race (c0's transfer otherwise starts at 1350, just before the
# trigger fires at ~1415).
SP_PRE_NOPS = 2
# weighted-reduction pieces: (row_tile, col0, col1, engine); engine is
#   dve    - STT reading PSUM directly
#   dve_sb - STT reading the ACT-made SBUF copy (drops the PSUM penalty)
#   pool   - STT on GPSIMD (SBUF copy only; GPSIMD cannot access PSUM)
# Pool pieces accumulate into a separate acc_p tile merged into acc by one
# DVE copy: if Pool STTs wrote acc directly, the store-scatter prep's
# desc-gen (1us on the Pool engine) would be ordered after them and land on
# the critical tail.
# 5th field: sched_ns - logical not-before time handed to the Tile
# scheduler (tile_wait_until) so the in-order DVE queue consumes pieces in
# data-arrival order instead of the scheduler's heap order.
RED_PIECES = [(0, 0, 640, "dve", 10010),
              (1, 0, 640, "dve", 10020),
              (2, 0, 640, "dve", 10030),
              (3, 0, 640, "dve_sb", 10040)]
# box-matmul row-tile order (PSUM r becomes ready in this order)
MM_ORDER = [0, 1, 2, 3]
# box-matmul / bias-fill col regions per row tile (PSUM accumulation groups;
# each region's stop releases its reduction pieces); pieces must not
# straddle region boundaries of their row.
MM_REGIONS = {0: [(0, 512), (512, 640)],
              1: [(0, 512), (512, 640)],
              2: [(0, 512), (512, 640)],
              3: [(0, 512), (512, 640)]}
# compare-bank build order: ('r'|'c', lo, hi) pieces; first box matmul needs
# cmpr[0:RP] and cmpc[0:<first r0 region end>].
CMP_PIECES = [("r", 0, RP), ("c", 0, 512), ("c", 512, W), ("r", RP, H)]
BIAS_COLS = 512     # bias matmul chunk width (ones tile width)
FILLER_N = 0 
GATE_DELAY_COLS = 0  # sizes the DVE delay before the loss-gate memset       # PE-warming refill matmuls between bias fills and mms
MERGE_BEFORE_LAST = True  # emit the acc_p merge before the last DVE piece

_compiled_cache: dict[int, "bass.Bass"] = {}
_TRACE = False      # set True (e.g. from test.py) to capture a HW profile
_last_bkr = None    # last BassKernelResults

N_ACC = len(RED_PIECES)


def _build_kernel(n_groups: int) -> "bass.Bass":
    """Builds the per-core Bass kernel; n_groups = ceil(boxes_per_image/32)."""
    # Bass.__init__ emits const-AP memsets (4 Pool ops, ~380ns) plus an
    # all-engine barrier before any kernel instruction. This kernel never
    # reads the const APs (activation(func=Copy) keeps float bias/scale as
    # immediates), and the Pool queue must reach the SWDGE preps as early
    # as possible - skip both during init only.
    _orig_barrier = bass.Bass.all_engine_barrier
    _orig_memset = bass.BassGpSimd.memset
    bass.Bass.all_engine_barrier = lambda self, *, sem_only=False: None
    bass.BassGpSimd.memset = lambda self, ap, constant: None
    try:
        nc = bacc.Bacc("TRN2", target_bir_lowering=False, debug=False,
                       num_devices=N_CORES)
    finally:
        bass.Bass.all_engine_barrier = _orig_barrier
        bass.BassGpSimd.memset = _orig_memset
    dt = mybir.dt
    loss_d = nc.dram_tensor("loss", [H, W], dt.float32, kind="ExternalInput")
    # bounds layout [128, 64] f32 (rows padded to 256B for the SWDGE
    # gather); for group g:
    #   col 2g   : row bounds [v1,v1,v2,v2] x32 (banks A,A,B,B)
    #   col 2g+1 : col bounds [u1,u2,u1,u2] x32 (banks C,D,C,D)
    # col 2G = row signs [+12,+12,-12,-12]x32 ; col 2G+1 = [+1,-1,+1,-1]x32
    bounds_d = nc.dram_tensor("bounds", [128, 64], dt.float32,
                              kind="ExternalInput")
    # acc output rows padded to 64 f32 (256B) so the SWDGE scatter's row
    # stride meets the 256B-multiple requirement; host reads [:, :N_ACC].
    acc_d = nc.dram_tensor("acc", [128, 64], dt.float32,
                           kind="ExternalOutput")

    with TileContext(nc) as tc:
        with (
            tc.tile_pool(name="const", bufs=1) as cpool,
            tc.tile_pool(name="lbuf", bufs=1) as lpool,
            tc.tile_pool(name="mbuf", bufs=2 * n_groups) as mpool,
            tc.tile_pool(name="sbuf", bufs=1) as spool,
            tc.tile_pool(name="psum", bufs=1, space="PSUM") as psum,
        ):
            # identity SWDGE indices (idx i at [i%16, i//16]), written as
            # immediate data by the otherwise-idle ACT sequencer so neither
            # SP (loss DMA issue) nor Pool (desc-gen) pays for it.
            # bounds DMA first on the HWDGE: its completion sem gates the
            # cmp -> matmul -> PSUM chain. (A SWDGE gather prep+trigger for
            # the bounds would free the issue slot, but the GPSIMD gather
            # ucode misreads its index table under this runtime - verified
            # wrong data - so the bounds ride the plain HWDGE path.)
            bt = cpool.tile([128, 64], dt.float32, tag="bounds")
            nc.sync.dma_start(out=bt[:], in_=bounds_d[:])

            # identity scatter indices for the SWDGE store: the GPSIMD ucode
            # reads a [16, num/16] int16 block (slot i at [i%16, i//16])
            # REPLICATED across the 8 ucode cores' partition groups, i.e.
            # idxs[p, s] = (p % 16) + 16*s on all 128 partitions. Verified
            # identity on the runtime via a scatter roundtrip.
            idxs = cpool.tile([128, 8], dt.int16, tag="sidx")
            iramp = cpool.tile([128, 8], dt.int16, tag="iramp")
            nc.gpsimd.iota(iramp[:], pattern=[[0, 8]], base=0,
                           channel_multiplier=1,
                           allow_small_or_imprecise_dtypes=True)
            nc.gpsimd.iota(idxs[:], pattern=[[16, 8]], base=0,
                           channel_multiplier=0,
                           allow_small_or_imprecise_dtypes=True)
            nc.vector.tensor_scalar(out=iramp[:], in0=iramp[:], scalar1=15,
                                    scalar2=None,
                                    op0=mybir.AluOpType.bitwise_and)
            nc.vector.tensor_tensor(out=idxs[:], in0=idxs[:], in1=iramp[:],
                                    op=mybir.AluOpType.add)
            nidx_reg = nc.gpsimd.to_reg(128)

            # SWDGE store path: a prepare_only scatter-add whose descriptors
            # are built NOW on Pool; the data read of acc is deferred to the
            # trigger_dma at the end (Tile moves the RAW deps there), which
            # skips the HWDGE issue slot + DGE latency on the critical tail.
            acc = cpool.tile([128, N_ACC], dt.float32, tag="acc")
            store_sem = nc.alloc_semaphore("acc_scatter_dma")
            nc.gpsimd.dma_scatter_add(
                acc_d[:, 0:N_ACC],
                acc[:].unsqueeze(1),
                idxs[:],
                128, nidx_reg,
                N_ACC,
                elem_step=64,
                prepare_only=True,
                sem=store_sem,
            )

            # Gate the loss chunks' HWDGE issues behind a tiny DVE memset
            # (WAW on one column of every chunk): without it, c0's transfer
            # starts at 1350 and the bounds gather-trigger (~1415) queues
            # behind the whole loss stream. The ~220ns release puts c0's
            # transfer right after the bounds transfer instead. Must be the
            # FIRST DVE op so the release lands at ~220.

            lt = lpool.tile([RP, N_RT * W], dt.float32, tag="loss")
            # ones row for the K=1 bias matmuls
            ones = cpool.tile([1, BIAS_COLS], dt.bfloat16, tag="ones")
            with tc.tile_wait_until(0.0005):
                nc.vector.memset(ones[:], 1.0)

            # acc is 128 partitions (scatter tokens cover all 128); rows
            # RP..127 are memset-zeroed and land in ignored DRAM rows.
            nc.vector.memset(acc[:], 0.0)

            for (r, c0, c1) in LOSS_CHUNKS:
                nc.sync.dma_start(
                    out=lt[:, r * W + c0:r * W + c1],
                    in_=loss_d[r * RP:(r + 1) * RP, c0:c1])

            # iota replacement: Pool is busy with SWDGE desc-gen, so build
            # io = [0..W) per partition on DVE with a +1 prefix scan over a
            # broadcast const-1 column (fp32 state, exact integers).
            const1 = cpool.tile([128, 1], dt.float32, tag="const1")
            nc.vector.memset(const1[:], 1.0)
            io = cpool.tile([128, W], dt.float32, tag="iota")
            ones_b = const1[:].to_broadcast([128, W])
            nc.vector.tensor_tensor_scan(
                out=io[:], data0=ones_b, data1=ones_b, initial=-1.0,
                op0=mybir.AluOpType.add, op1=mybir.AluOpType.bypass)

            # one PSUM tile per (row, region) - 8 single-bank tiles. Tile
            # serializes PSUM readers of the SAME tile (an ACT copy would
            # gate a later DVE read of the sibling region by ~600ns);
            # per-region tiles keep the readers independent.
            # rows whose pieces all sit inside one region get per-region
            # tiles (so same-row pieces/copies never share a PSUM tile and
            # never serialize); rows with a region-spanning piece keep one
            # two-bank tile.
            def contained(r):
                return all(any(c0 >= k0 and c1 <= k1
                               for (k0, k1) in MM_REGIONS[r])
                           for (rr, c0, c1, _e, _w) in RED_PIECES if rr == r)
            psum_tiles = {}
            for r in range(N_RT):
                if contained(r):
                    for (k0, k1) in MM_REGIONS[r]:
                        psum_tiles[(r, k0, k1)] = psum.tile(
                            [RP, k1 - k0], dt.float32, tag=f"cnt{r}_{k0}",
                            name=f"cnt{r}_{k0}")
                else:
                    t = psum.tile([RP, 1024], dt.float32, tag=f"cnt{r}",
                                  name=f"cnt{r}")
                    psum_tiles[(r, 0, W)] = t

            def seg_slice(r, c0, c1):
                if (r, 0, W) in psum_tiles:
                    return psum_tiles[(r, 0, W)][:, c0:c1]
                for (k0, k1) in MM_REGIONS[r]:
                    if c0 >= k0 and c1 <= k1:
                        return psum_tiles[(r, k0, k1)][:, c0 - k0:c1 - k0]
                raise AssertionError((r, c0, c1))

            def mm_chunks(r):
                return MM_REGIONS[r]

            # Regions whose cnt' is read straight from PSUM by a DVE piece
            # need the +1 bias pre-filled by PE; regions consumed only
            # through ACT copies get the +1 as the copy's bias instead
            # (saves PE fill time). A region is filled iff some dve piece
            # overlaps it; pieces must not mix fill-state within one ACT
            # copy row, so bias bookkeeping is per REGION.
            def regions_of(r, c0, c1):
                out = [(r, k0, k1) for (k0, k1) in MM_REGIONS[r]
                       if k0 < c1 and c0 < k1]
                assert out, (r, c0, c1)
                return out
            filled_regions = sorted({rg
                                     for (r, c0, c1, e, _w) in RED_PIECES
                                     if e == "dve"
                                     for rg in regions_of(r, c0, c1)})
            filled_rows = sorted({r for (r, _, _) in filled_regions})
            # PE warmup: a tiny matmul eats the cold-pipeline penalty, then
            # bias fills (PSUM := 1.0) exactly covering the box matmul chunk
            # regions (misaligned start=True fills lose the bias under the
            # accumulation-group semantics).
            wr, wk, _ = filled_regions[0] if filled_regions else (0, 0, 0)
            nc.tensor.matmul(seg_slice(wr, wk, wk + 1), lhsT=ones[:, 0:RP],
                             rhs=ones[:, 0:1], start=True, stop=False,
                             skip_group_check=True)
            for (r, k, k1) in filled_regions:
                nc.tensor.matmul(
                    seg_slice(r, k, k1),
                    lhsT=ones[:, 0:RP], rhs=ones[:, 0:k1 - k],
                    start=True, stop=False, skip_group_check=True)
            # PE-warming refills: keep PE busy between the fills and the
            # cmp-gated box matmuls so the mms run at a hot pstate; each
            # refill re-asserts the bias on a filled region (writes 1.0).
            if filled_regions:
                fr, fk, fk1 = filled_regions[-1]
                for _ in range(FILLER_N):
                    nc.tensor.matmul(
                        seg_slice(fr, fk, fk1),
                        lhsT=ones[:, 0:RP], rhs=ones[:, 0:fk1 - fk],
                        start=True, stop=False, skip_group_check=True)

            # per-group signed compare banks (DVE; 2x_2p mode, all SBUF)
            sgr = bt[:, 2 * n_groups:2 * n_groups + 1]
            sgc = bt[:, 2 * n_groups + 1:2 * n_groups + 2]
            cmprs, cmpcs = [], []
            for g in range(n_groups):
                cmpc = mpool.tile([128, W], dt.bfloat16, tag="cmpc",
                                  name=f"cmpc{g}")
                cmpr = mpool.tile([128, H], dt.bfloat16, tag="cmpr",
                                  name=f"cmpr{g}")

                def cmp_op(tile, lo, hi, bcol, sg):
                    nc.vector.tensor_scalar(
                        out=tile[:, lo:hi], in0=io[:, lo:hi],
                        scalar1=bt[:, bcol:bcol + 1], scalar2=sg,
                        op0=mybir.AluOpType.is_ge, op1=mybir.AluOpType.mult)
                for (kind, lo, hi) in CMP_PIECES:
                    if kind == "r":
                        cmp_op(cmpr, lo, hi, 2 * g, sgr)
                    else:
                        cmp_op(cmpc, lo, hi, 2 * g + 1, sgc)
                cmprs.append(cmpr)
                cmpcs.append(cmpc)

            # box matmuls accumulate 12*cnt on top of the bias fill; for
            # regions with no fill (consumed via ACT copies that add the +1
            # as bias) the first group's matmul must start the accumulation
            # group itself, or it accumulates onto stale PSUM.
            for r in MM_ORDER:
                r0 = r * RP
                for (c0, c1) in mm_chunks(r):
                    first_starts = (r, c0, c1) not in filled_regions
                    for g in range(n_groups):
                        nc.tensor.matmul(
                            seg_slice(r, c0, c1),
                            lhsT=cmprs[g][:, r0:r0 + RP],
                            rhs=cmpcs[g][:, c0:c1],
                            start=(first_starts and g == 0),
                            stop=(g == n_groups - 1),
                            skip_group_check=True)

            # ACT copies cnt' PSUM ranges needed by pool/dve_sb pieces to
            # SBUF, early and off the critical path (gated only on PSUM).
            # Merge adjacent non-dve pieces per row tile into one copy; the
            # copy adds the +1 bias for rows PE did not pre-fill.
            w_sb = spool.tile([RP, N_RT * W], dt.float32, tag="wsb")
            copies = []
            for (r, c0, c1, eng, _w) in RED_PIECES:
                if eng == "dve":
                    continue
                if copies and copies[-1][0] == r and copies[-1][2] == c0:
                    copies[-1] = (r, copies[-1][1], c1)
                else:
                    copies.append((r, c0, c1))
            for (r, c0, c1) in copies:
                nc.scalar.activation(
                    out=w_sb[:, r * W + c0:r * W + c1],
                    in_=seg_slice(r, c0, c1),
                    func=mybir.ActivationFunctionType.Copy,
                    bias=0.0 if all(rg in filled_regions
                                    for rg in regions_of(r, c0, c1))
                    else 1.0)

            # fused weighted reductions:
            #   acc[:RP, i] = sum_cols (min(cnt', 13) * loss_piece)
            # DVE pieces accumulate into acc cols [0, n_dve); Pool pieces
            # into acc_p, merged into acc's tail cols by one DVE copy (host
            # sums all columns, order irrelevant). scr is a garbage sink;
            # 1024-col slots per piece so the subtile dep tracker never
            # serializes pieces across engines.
            scr = spool.tile([RP, (N_ACC + 2) * 1024], dt.float32,
                             tag="scr")

            def emit_piece(i, r, c0, c1, eng, acc_out, scr_col):
                if eng == "dve":
                    in0 = seg_slice(r, c0, c1)
                else:
                    in0 = w_sb[:, r * W + c0:r * W + c1]
                engine = nc.gpsimd if eng == "pool" else nc.vector
                engine.scalar_tensor_tensor(
                    out=scr[:, scr_col:scr_col + (c1 - c0)],
                    in0=in0,
                    scalar=FG_WEIGHT,
                    in1=lt[:, r * W + c0:r * W + c1],
                    op0=mybir.AluOpType.min, op1=mybir.AluOpType.mult,
                    accum_out=acc_out)

            # the tile_wait_until stamps force the scheduler to keep each
            # in-order engine queue in emission (= data-arrival) order; the
            # values are schedule-time hints only (TimelineSim re-times).
            for i, (r, c0, c1, eng, w_ns) in enumerate(RED_PIECES):
                with tc.tile_wait_until(w_ns * 1e-6, enable=w_ns > 0):
                    emit_piece(i, r, c0, c1, eng, acc[0:RP, i:i + 1],
                               i * 1024)

            # fire the prepped scatter; Tile gates this on the reductions'
            # acc writes (deferred RAW) so it goes straight to the DMA
            # engines when the last accum lands.
            nc.gpsimd.trigger_dma(count=None)

        # The TileContext exit emits drain -> barrier -> sem-clear -> barrier.
        # The drain plus the first barrier (which orders the sem-clear after
        # all engines quiesce) must stay; the trailing barrier only
        # re-rendezvouses already-idle queues, so skip it (~250ns off the
        # tail). Skipping both breaks NEFF execution (verified).
        _exit_calls = [0]

        def _barrier_once(self, *, sem_only=False):
            _exit_calls[0] += 1
            if _exit_calls[0] == 1:
                return _orig_barrier(self, sem_only=sem_only)
            return None

        bass.Bass.all_engine_barrier = _barrier_once
    bass.Bass.all_engine_barrier = _orig_barrier

    # Tile books each SWDGE prep on a DMASW lane (teardown waits
    # DMASW{k} >= 16) but leaves the user sem from sem= in OnUpdate[0],
    # which is what walrus encodes into the descriptor and what the
    # trigger replay bumps - so the lane sem would never move. Point the
    # preps' OnUpdate[0] at their lane sems (in lane order = emission
    # order) before codegen.
    fn = nc.m.functions[0]
    lane_sems: dict[str, int] = {}
    preps = []
    for blk in fn.blocks:
        for ins in blk.instructions:
            si = ins.sync_info
            if si is None:
                continue
            for w in si.on_wait:
                if w.ant_name and w.ant_name.startswith("DMASW"):
                    lane_sems[w.ant_name] = w.id
            if type(ins).__name__ in ("InstDMAScatterAddAnt",
                                      "InstDMAGatherAnt"):
                preps.append(ins)
    for ins, lane_name in zip(preps, sorted(lane_sems)):
        u0 = ins.sync_info.on_update[0]
        u0.id = lane_sems[lane_name]
        u0.ant_name = lane_name
    assert len(preps) == len(lane_sems), (len(preps), lane_sems)

    # The store scatter's completion sem (the LAST DMASW lane) fires ~1us
    # after every other DMA sem, but the SP teardown processes its quiesce
    # waits serially at ~46ns each AFTER the blocking one resolves. Move
    # the store-lane wait into the LAST quiesce EventSemaphore so the
    # earlier waits all resolve beforehand and the final barrier starts
    # ~230ns sooner.
    store_lane = sorted(lane_sems)[-1]
    for blk in fn.blocks:
        for ins in blk.instructions:
            si = ins.sync_info
            if (str(ins.engine) != "EngineType.SP" or si is None
                    or not any(w.ant_name == store_lane
                               for w in si.on_wait)
                    or len(si.on_wait) < 2):
                continue
            ws = si.on_wait
            ws.sort(key=lambda w: w.ant_name == store_lane)
            si.on_wait = ws

    nc.compile()
    return nc


def _box_bounds(boxes: np.ndarray) -> np.ndarray:
    """[n,4] float boxes -> integer bounds (u1,v1,u2,v2) in feature coords.

    Degenerate boxes (hi <= lo on either axis after floor/ceil) rasterize to
    an empty mask in the reference; zero them so the signed-bank expansion
    (A-B)(C-D) also contributes exactly nothing instead of going negative.
    """
    b = boxes.astype(np.float64) / DOWNSAMPLE
    u1 = np.floor(b[:, 0])
    v1 = np.floor(b[:, 1])
    u2 = np.ceil(b[:, 2])
    v2 = np.ceil(b[:, 3])
    out = np.stack([u1, v1, u2, v2], axis=1)
    empty = (u2 <= u1) | (v2 <= v1)
    out[empty] = 0.0
    return out.astype(np.float32)


def kernel(loss: np.ndarray, gt_boxes2d: np.ndarray,
           num_gt_per_img: np.ndarray) -> np.ndarray:
    loss = np.ascontiguousarray(np.asarray(loss, dtype=np.float32))
    boxes = np.asarray(gt_boxes2d, dtype=np.float32).reshape(-1, 4)
    counts = np.asarray(num_gt_per_img).astype(np.int64)
    t_total = boxes.shape[0]

    # replicate jnp.repeat(arange(B), counts, total_repeat_length=T):
    # gather via segment-end search; positions past sum(counts) clip to the
    # LAST array element (image B-1), verified against jax on CPU.
    ends = np.cumsum(np.clip(counts, 0, None))
    bids = np.clip(np.searchsorted(ends, np.arange(t_total), side="right"),
                   0, B - 1)

    per_img = [np.nonzero(bids == b)[0] for b in range(B)]
    max_n = max((len(ix) for ix in per_img), default=0)
    n_groups = max(1, -(-max_n // 32))

    nc = _compiled_cache.get(n_groups)
    if nc is None:
        nc = _build_kernel(n_groups)
        _compiled_cache[n_groups] = nc

    scale = FG_WEIGHT - BG_WEIGHT  # 12: folded into the row-bank signs
    sign_row = np.repeat([scale, scale, -scale, -scale], 32).astype(np.float32)
    sign_col = np.repeat([1.0, -1.0, 1.0, -1.0], 32).astype(np.float32)

    in_maps = []
    assert 2 * n_groups + 2 <= 64, "bounds exceed the 64-col padded layout"
    for b in range(B):
        bb = _box_bounds(boxes[per_img[b]])  # [n_b, 4] = (u1, v1, u2, v2)
        bounds = np.zeros((128, 64), dtype=np.float32)
        bounds[:, 2 * n_groups] = sign_row
        bounds[:, 2 * n_groups + 1] = sign_col
        for g in range(n_groups):
            chunk = bb[g * 32:(g + 1) * 32]
            k = chunk.shape[0]
            if k == 0:
                continue
            rowv = bounds[:, 2 * g]
            colv = bounds[:, 2 * g + 1]
            rowv[0:k] = chunk[:, 1]        # A: v1
            rowv[32:32 + k] = chunk[:, 1]  # A: v1
            rowv[64:64 + k] = chunk[:, 3]  # B: v2
            rowv[96:96 + k] = chunk[:, 3]  # B: v2
            colv[0:k] = chunk[:, 0]        # C: u1
            colv[32:32 + k] = chunk[:, 2]  # D: u2
            colv[64:64 + k] = chunk[:, 0]  # C: u1
            colv[96:96 + k] = chunk[:, 2]  # D: u2
        in_maps.append({"loss": loss[b], "bounds": bounds})

    global _last_bkr
    _last_bkr = run_bass_kernel_spmd(nc, in_maps, list(range(N_CORES)),
                                     trace=_TRACE)
    results = _last_bkr.results

    total = np.float64(0.0)
    for b in range(B):
        total += results[b]["acc"][:RP, :N_ACC].astype(np.float64).sum()
    out = total / (B * H * W)
    return np.asarray(out, dtype=np.float32)
